# revision 2
# baseline (speedup 1.0000x reference)
"""Trainium2 Bass kernel for nn_Actor_att1 (gnn_message_passing).

Data-parallel over 8 NeuronCores: each core processes B/8 = 32768 rows.

Per-core pipeline (transposed activation layout [feature, batch], tiles of 512):
  - L1 of all 32 encoders (self + 15 other + 16 food) as ONE block-diagonal
    matmul group: W1_big [127, 1024], 8 matmuls of [127,128]x[127,512].
  - L2 similarly block-diagonal: 8 matmuls [128,64] -> enc_T [512 feat, 512 b].
  - Attention without softmax normalization: LayerNorm is scale-invariant, so
    unnormalized weights w_n = exp(score_n/4) suffice.  Score reduction over
    d (partition dim) and the weighted sums over n are PE "ones-matmuls";
    mean-centering of the attended vector is folded into the reduction matrix
    (RepC = blockdiag(I - 11^T/16)), so variance is just mean(C^2).
  - rstd = sqrt(1/(var+eps)) computed in a separate phase (ACT Sqrt lives in a
    different table set than Exp/Tanh -> 3 phases, 2 table switches total).
  - Final MLP in transposed layout, leaky-relu via ACT Lrelu, tanh via ACT.
  - Output transposed back to natural via PE, packed [128, 512] per core,
    un-permuted on the host.
"""

import numpy as np
import ml_dtypes

import concourse.bass as bass
import concourse.tile as tile
from concourse import mybir
from concourse.bass_utils import run_bass_kernel_spmd

F32 = mybir.dt.float32
BF16 = mybir.dt.bfloat16

N_CORES = 8
B_FULL = 262144
BC = B_FULL // N_CORES      # 32768 rows per core
OBS = 127
TB = 512                    # batch tile
NT = BC // TB               # 64 tiles
NSUB = 4                    # 128-row subtiles per tile
EPS = 1e-5

# ---- CONSTF32 column layout ----
W1_C = 0            # [0:127, 0:1024]  block-diag L1 weights
EYE_C = 1024        # [128,128] f32 identity
B1BIG_C = 1152      # 8 cols, [128,1] each: L1 bias per block
B2SB_C = 1160       # 4 cols: L2 bias per psum pair
B1M_C = 1164        # rows 0:32  final-MLP b1
B2M_C = 1165        # rows 0:32  final-MLP b2
B3M_C = 1166        # rows 0:2   final-MLP b3
F32_COLS = 1168

# ---- CONSTB (bf16) column layout ----
W2_C = 0            # [128, 1024]: 8 split-K blocks of [128,128]
EYEB_C = 1024       # [128,128] bf16 identity
SCORE_C = 1152      # 4 blocks [128,32]
REPC_C = 1280       # 4 blocks [128,32]
REPSELF_C = 1408    # [0:16, 128]
REPW_C = 1536       # 4 blocks [0:32, 128]
SQONES_C = 2048     # [0:32, 2]
M1SELF_C = 2050     # [0:16, 32]
M1REST_C = 2082     # [0:32, 32]
MW2_C = 2114        # [0:32, 32]
MW3_C = 2146        # [0:32, 2]
REPC48_C = 2176     # 4 blocks [128,48]: centered numerators + self identity
BF_COLS = 2368

_BASS_CACHE = {}


def _pack_consts(p):
    """Host-side packing of all weights into two constant arrays."""
    cf = np.zeros((128, F32_COLS), np.float32)
    cb = np.zeros((128, BF_COLS), np.float32)

    # --- W1 block-diag [127, 1024] + b1big [1024] ---
    w1 = np.zeros((127, 1024), np.float32)
    b1 = np.zeros(1024, np.float32)
    # agent 0: self  (input cols 0:4)
    w1[0:4, 0:32] = p['en_w1']
    b1[0:32] = p['en_b1']
    for i in range(15):               # other agents, input col map
        c = 32 + 32 * i
        w1[4 + 2 * i, c:c + 32] = p['oa_w1'][0]
        w1[5 + 2 * i, c:c + 32] = p['oa_w1'][1]
        w1[34 + 2 * i, c:c + 32] = p['oa_w1'][2]
        w1[35 + 2 * i, c:c + 32] = p['oa_w1'][3]
        w1[64 + i, c:c + 32] = p['oa_w1'][4]
        b1[c:c + 32] = p['oa_b1']
    for j in range(16):               # food agents
        c = 512 + 32 * j
        for k in range(3):
            w1[79 + 3 * j + k, c:c + 32] = p['g_w1'][k]
        b1[c:c + 32] = p['g_b1']
    cf[0:127, W1_C:W1_C + 1024] = w1
    cf[0:128, EYE_C:EYE_C + 128] = np.eye(128, dtype=np.float32)
    cf[:, B1BIG_C:B1BIG_C + 8] = b1.reshape(8, 128).T

    # --- W2 block-diag: 8 blocks [128, 64] ---
    w2s = [p['en_w2']] + [p['oa_w2']] * 15 + [p['g_w2']] * 16
    b2s = [p['en_b2']] + [p['oa_b2']] * 15 + [p['g_b2']] * 16
    w2big = np.zeros((128, 1024), np.float32)
    b2big = np.zeros(512, np.float32)
    for a in range(32):
        g, al = a // 4, a % 4        # g = h1 block, al = agent-in-block
        jj = a // 8                   # psum pair
        w2big[32 * al:32 * al + 32,
              128 * g + 16 * (a - 8 * jj):128 * g + 16 * (a - 8 * jj) + 16] = w2s[a]
        b2big[16 * a:16 * a + 16] = b2s[a]
    cb[:, W2_C:W2_C + 1024] = w2big
    cf[:, B2SB_C:B2SB_C + 4] = b2big.reshape(4, 128).T
    cb[:, EYEB_C:EYEB_C + 128] = np.eye(128, dtype=np.float32)

    # --- attention matrices, per feature-block j (agents 8j..8j+7) ---
    # score col for agent a: other (1..15) -> a-1 ; food (16..31) -> a-16+16
    for j in range(4):
        so = np.zeros((128, 32), np.float32)
        rc = np.zeros((128, 32), np.float32)
        rw = np.zeros((32, 128), np.float32)
        for nl in range(8):
            a = 8 * j + nl
            if a == 0:
                continue
            col = (a - 1) if a < 16 else (16 + a - 16)
            t = 0 if a < 16 else 1
            so[16 * nl:16 * nl + 16, col] = 1.0
            rw[col, 16 * nl:16 * nl + 16] = 1.0
            blk = np.eye(16, dtype=np.float32) - 1.0 / 16.0
            rc[16 * nl:16 * nl + 16, 16 * t:16 * t + 16] = blk
        cb[:, SCORE_C + 32 * j:SCORE_C + 32 * j + 32] = so
        cb[:, REPC_C + 32 * j:REPC_C + 32 * j + 32] = rc
        rc48 = np.zeros((128, 48), np.float32)
        rc48[:, 0:32] = rc
        if j == 0:
            rc48[np.arange(16), 32 + np.arange(16)] = 1.0  # self passthrough
            # score row 15 is never written -> exp(0)=1: route it to the
            # self rows of w_rep so products2 carries E_0's self unweighted
            rw[15, 0:16] = 1.0
        cb[:, REPC48_C + 48 * j:REPC48_C + 48 * j + 48] = rc48
        cb[0:32, REPW_C + 128 * j:REPW_C + 128 * j + 128] = rw
    rs = np.zeros((16, 128), np.float32)
    for k in range(8):
        rs[np.arange(16), 16 * k + np.arange(16)] = 1.0
    cb[0:16, REPSELF_C:REPSELF_C + 128] = rs
    sq = np.zeros((32, 2), np.float32)
    sq[0:16, 0] = 1.0 / 16.0
    sq[16:32, 1] = 1.0 / 16.0
    cb[0:32, SQONES_C:SQONES_C + 2] = sq

    # --- final MLP ---
    m_w1 = p['m_w1']  # [48, 32]; merged order [self, food, other]
    cb[0:16, M1SELF_C:M1SELF_C + 32] = m_w1[0:16]
    # M rows: 0-15 = other, 16-31 = food, 32-47 = self
    cb[0:48, M1REST_C:M1REST_C + 32] = np.concatenate(
        [m_w1[32:48], m_w1[16:32], m_w1[0:16]], axis=0)
    cb[0:32, MW2_C:MW2_C + 32] = p['m_w2']
    cb[0:32, MW3_C:MW3_C + 2] = p['m_w3']
    cf[0:32, B1M_C] = p['m_b1']
    cf[0:32, B2M_C] = p['m_b2']
    cf[0:2, B3M_C] = p['m_b3']

    # LN gain/bias are ones/zeros in setup_inputs; fold general case anyway:
    # out = relu(LN * g + b). We only support g==1, b==0 fast path; otherwise
    # fall back by folding g into rstd-mult (g per-dim requires a tensor op we
    # skip).  Assert instead.
    for k in ('oa_g', 'g_g'):
        assert np.allclose(p[k], 1.0), "LN gain != 1 unsupported"
    for k in ('oa_bln', 'g_bln'):
        assert np.allclose(p[k], 0.0), "LN bias != 0 unsupported"

    return cf, cb.astype(ml_dtypes.bfloat16)


def _split_multi_waits(nc):
    """This walrus build accepts only one sync-wait per instruction; move
    extra waits onto dedicated EventSemaphore instructions just before."""
    f = nc.m.functions[0]
    ctr = 0
    for blk in f.blocks:
        new_ins = []
        for ins in blk.instructions:
            si = getattr(ins, 'sync_info', None)
            ow = list(si.on_wait) if si is not None and si.on_wait else []
            if len(ow) > 1:
                for w in ow[:-1]:
                    ev = mybir.InstEventSemaphore(
                        name=f"wsplit_{ctr}",
                        engine=ins.engine,
                        ins=[], outs=[],
                        sync_info=mybir.SyncInfo(on_wait=[w], on_update=[]),
                    )
                    ctr += 1
                    new_ins.append(ev)
                si.on_wait = ow[-1:]
            new_ins.append(ins)
        blk.instructions[:] = new_ins
    return ctr


def _build_bass(nt=NT):
    nc = bass.Bass()
    s_in = nc.dram_tensor("s_in", [OBS, BC], F32, kind="ExternalInput")
    cfd = nc.dram_tensor("constf", [128, F32_COLS], F32, kind="ExternalInput")
    cbd = nc.dram_tensor("constb", [128, BF_COLS], BF16, kind="ExternalInput")
    out = nc.dram_tensor("out", [2, NT * TB], F32, kind="ExternalOutput")

    with tile.TileContext(nc) as tc:
        with (
            tc.tile_pool(name="singles", bufs=1) as singles,
            tc.tile_pool(name="xt", bufs=2) as xt_p,
            tc.tile_pool(name="h1", bufs=2) as h1_p,
            tc.tile_pool(name="enc", bufs=3) as enc_p,
            tc.tile_pool(name="work", bufs=3) as work_p,
            tc.tile_pool(name="pL", bufs=3, space="PSUM") as pL,
            tc.tile_pool(name="psm", bufs=1, space="PSUM") as psm
            , tc.tile_pool(name="p3", bufs=2, space="PSUM") as p3_p,
            tc.tile_pool(name="pacc", bufs=2, space="PSUM") as pacc,
        ):
            CF = singles.tile([128, F32_COLS], F32)
            CB = singles.tile([128, BF_COLS], BF16)
            nc.sync.dma_start(out=CF, in_=cfd[:, :])
            nc.sync.dma_start(out=CB, in_=cbd[:, :])
            eye = CF[:, EYE_C:EYE_C + 128]
            eyeb = CB[:, EYEB_C:EYEB_C + 128]

            # PE warm-up: make PE observe the const DMAs once, so later
            # matmuls carry at most one (fresh) DMA sync-wait each -- walrus
            # rejects Matmults with 2+ sync waits.
            scratch = singles.tile([1, 48], F32)
            dscratch = singles.tile([1, 8], F32)
            wf = psm.tile([128, 128], F32, tag="sm")
            nc.tensor.transpose(wf[0:128, 0:128], eye, eye)
            nc.vector.tensor_copy(out=scratch[0:1, 0:8], in_=wf[0:1, 0:8])
            wb = psm.tile([128, 128], BF16, tag="sm")
            nc.tensor.transpose(wb[0:128, 0:128], eyeb, eyeb)
            nc.vector.tensor_copy(out=scratch[0:1, 8:16], in_=wb[0:1, 0:8])
            # every compute engine observes both const DMAs once, so
            # steady-state instructions carry few sync waits
            nc.scalar.copy(out=scratch[0:1, 16:24], in_=CF[0:1, 0:8])
            nc.scalar.copy(out=scratch[0:1, 24:32], in_=CB[0:1, 0:8])
            nc.vector.tensor_copy(out=scratch[0:1, 32:40], in_=CF[0:1, 0:8])
            nc.vector.tensor_copy(out=scratch[0:1, 40:48], in_=CB[0:1, 0:8])

            rn_stage = singles.tile([128, NT * 192], BF16)
            var_stage = singles.tile([128, NT * 8], F32)
            rstd_stage = singles.tile([128, NT * 8], F32)

            # phase-1/phase-3 software pipeline: first half of phase 1,
            # its rstd, then phase 1 (second half) interleaved with
            # phase 3 (first half); only the two Sqrt ops switch tables.
            def phase1_body(t, _st):
                r0 = t * TB
                if t % 2 == 0:
                    xT2 = xt_p.tile([127, 2 * TB], F32, tag="xT")
                    _st['xT2'] = xT2
                    # absorber: Pool observes the PE WAR tick so the DMA
                    # itself carries only its lane wait (HW allows 1)
                    nc.gpsimd.memset(xT2[0:1, 0:4], 0.0)
                    nc.gpsimd.dma_start(
                        out=xT2, in_=s_in[:, r0:r0 + 2 * TB])
                    xT = _st['xT2'][:, 0:TB]
                else:
                    xT = _st['xT2'][:, TB:2 * TB]

                # L1 + L2 block-diagonal encoders
                h1t = []
                for g in range(8):
                    ps = pL.tile([128, TB], F32, tag="mm")
                    nc.tensor.matmul(
                        ps, CF[0:127, W1_C + 128 * g:W1_C + 128 * (g + 1)],
                        xT, start=True, stop=True)
                    hg = h1_p.tile([128, TB], BF16, tag=f"h1{g}")
                    bias = CF[:, B1BIG_C + g:B1BIG_C + g + 1]
                    if g % 2 == 0:
                        nc.scalar.activation(
                            out=hg, in_=ps,
                            func=mybir.ActivationFunctionType.Relu,
                            bias=bias, scale=1.0)
                    else:
                        nc.vector.tensor_scalar(
                            out=hg, in0=ps, scalar1=bias, scalar2=0.0,
                            op0=mybir.AluOpType.add, op1=mybir.AluOpType.max)
                    h1t.append(hg)

                E = []
                for jj in range(4):
                    ps = pL.tile([128, TB], F32, tag="mm")
                    for half in range(2):
                        g = 2 * jj + half
                        nc.tensor.matmul(
                            ps,
                            CB[:, W2_C + 128 * g:W2_C + 128 * (g + 1)],
                            h1t[g], start=(half == 0), stop=(half == 1))
                    ej = enc_p.tile([128, TB], BF16, tag=f"E{jj}")
                    bias = CF[:, B2SB_C + jj:B2SB_C + jj + 1]
                    if jj % 2 == 0:
                        nc.scalar.activation(
                            out=ej, in_=ps,
                            func=mybir.ActivationFunctionType.Relu,
                            bias=bias, scale=1.0)
                    else:
                        nc.vector.tensor_scalar(
                            out=ej, in0=ps, scalar1=bias, scalar2=0.0,
                            op0=mybir.AluOpType.add, op1=mybir.AluOpType.max)
                    E.append(ej)

                # self replicated across the 8 16-row groups
                srp = pL.tile([128, TB], F32, tag="mm")
                nc.tensor.matmul(srp, CB[0:16, REPSELF_C:REPSELF_C + 128],
                                 E[0][0:16, :], start=True, stop=True)
                sr = work_p.tile([128, TB], BF16, tag="sr")
                nc.scalar.copy(out=sr, in_=srp)

                # scores -> S [32, 512]
                S = pacc.tile([32, TB], F32, tag="acc")
                Pj_list = []
                for jj in range(4):
                    pj = work_p.tile([128, TB], BF16, tag=f"P{jj}")
                    if jj < 2:
                        nc.vector.tensor_mul(pj, E[jj], sr)
                    else:
                        nc.gpsimd.tensor_mul(pj, E[jj], sr)
                    Pj_list.append(pj)
                for jj in range(4):
                    nc.tensor.matmul(
                        S, CB[:, SCORE_C + 32 * jj:SCORE_C + 32 * (jj + 1)],
                        Pj_list[jj], start=(jj == 0), stop=(jj == 3))

                # w = exp(score / 4)
                wt = work_p.tile([32, TB], BF16, tag="wt")
                nc.scalar.activation(out=wt, in_=S,
                                     func=mybir.ActivationFunctionType.Exp,
                                     scale=0.25)

                # centered numerators + self passthrough, C [48, 512]
                C = pacc.tile([48, TB], F32, tag="acc")
                P2_list = []
                for jj in range(4):
                    wr = pL.tile([128, TB], F32, tag="mm")
                    nc.tensor.matmul(
                        wr, CB[0:32, REPW_C + 128 * jj:REPW_C + 128 * (jj + 1)],
                        wt, start=True, stop=True)
                    p2 = work_p.tile([128, TB], BF16, tag=f"P2{jj}")
                    nc.vector.tensor_mul(p2, E[jj], wr)
                    P2_list.append(p2)
                for jj in range(4):
                    nc.tensor.matmul(
                        C, CB[:, REPC48_C + 48 * jj:REPC48_C + 48 * (jj + 1)],
                        P2_list[jj], start=(jj == 0), stop=(jj == 3))

                rsb = work_p.tile([48, TB], BF16, tag="rsb")
                nc.scalar.activation(out=rsb, in_=C,
                                     func=mybir.ActivationFunctionType.Relu)
                sqb = work_p.tile([48, TB], BF16, tag="sqb")
                nc.scalar.activation(out=sqb, in_=C,
                                     func=mybir.ActivationFunctionType.Square)

                # var [128, 2] per subtile; +EPS folded into the drain
                vn = psm.tile([128, 128], F32, tag="sm")
                for s in range(NSUB):
                    nc.tensor.matmul(
                        vn[:, 2 * s:2 * s + 2],
                        sqb[:, 128 * s:128 * (s + 1)],
                        CB[0:48, SQONES_C:SQONES_C + 2],
                        start=True, stop=True)
                nc.vector.tensor_scalar(
                    out=var_stage[:, 8 * t:8 * t + 8], in0=vn[:, 0:8],
                    scalar1=EPS, scalar2=None, op0=mybir.AluOpType.add)

                # transpose relu'd numerators (+self) to natural, stage
                rn = psm.tile([128, 192], BF16, tag="sm")
                for s in range(NSUB):
                    nc.tensor.transpose(
                        rn[:, 48 * s:48 * s + 48],
                        rsb[:, 128 * s:128 * (s + 1)], eyeb[0:48, 0:48])
                nc.vector.tensor_copy(
                    out=rn_stage[:, 192 * t:192 * (t + 1)], in_=rn)

            def phase3_body(t):
                # scale LN cols by rstd; self cols pass through unscaled
                mn = work_p.tile([128, 192], BF16, tag="mn")
                for s in range(NSUB):
                    rsl = rstd_stage[:, 8 * t + 2 * s:8 * t + 2 * s + 2]
                    rb = bass.AP(tensor=rsl.tensor, offset=rsl.offset,
                                 ap=[rsl.ap[0], rsl.ap[1], [0, 16]])
                    nc.gpsimd.tensor_mul(
                        mn[:, 48 * s:48 * s + 32].rearrange(
                            "p (t2 d) -> p t2 d", t2=2),
                        rn_stage[:, 192 * t + 48 * s:192 * t + 48 * s + 32
                                 ].rearrange("p (t2 d) -> p t2 d", t2=2),
                        rb)
                    nc.gpsimd.tensor_copy(
                        out=mn[:, 48 * s + 32:48 * s + 48],
                        in_=rn_stage[:, 192 * t + 48 * s + 32:
                                     192 * t + 48 * s + 48])
                # transpose back: MT [48, 512]
                mt = p3_p.tile([48, TB], BF16, tag="p3")
                for s in range(NSUB):
                    nc.tensor.transpose(
                        mt[:, 128 * s:128 * (s + 1)],
                        mn[:, 48 * s:48 * s + 48], eyeb)
                msb = work_p.tile([48, TB], BF16, tag="msb")
                nc.vector.tensor_copy(out=msb, in_=mt)

                # final MLP (merged = [other, food, self] rows of msb)
                h1f = p3_p.tile([32, TB], F32, tag="p3")
                nc.tensor.matmul(h1f, CB[0:48, M1REST_C:M1REST_C + 32], msb,
                                 start=True, stop=True)
                hh1 = work_p.tile([32, TB], BF16, tag="hh1")
                nc.scalar.activation(out=hh1, in_=h1f,
                                     func=mybir.ActivationFunctionType.Lrelu,
                                     bias=CF[0:32, B1M_C:B1M_C + 1],
                                     scale=1.0, alpha=0.01)
                h2f = p3_p.tile([32, TB], F32, tag="p3")
                nc.tensor.matmul(h2f, CB[0:32, MW2_C:MW2_C + 32], hh1,
                                 start=True, stop=True)
                hh2 = work_p.tile([32, TB], BF16, tag="hh2")
                nc.scalar.activation(out=hh2, in_=h2f,
                                     func=mybir.ActivationFunctionType.Lrelu,
                                     bias=CF[0:32, B2M_C:B2M_C + 1],
                                     scale=1.0, alpha=0.01)
                of = p3_p.tile([32, TB], F32, tag="p3")
                nc.tensor.matmul(of[0:2, :], CB[0:32, MW3_C:MW3_C + 2], hh2,
                                 start=True, stop=True)
                osb = work_p.tile([2, TB], F32, tag="osb")
                nc.scalar.activation(out=osb, in_=of[0:2, :],
                                     func=mybir.ActivationFunctionType.Tanh,
                                     bias=CF[0:2, B3M_C:B3M_C + 1], scale=1.0)

                nc.gpsimd.tensor_copy(out=dscratch[0:1, 0:4],
                                      in_=osb[0:1, 508:512])
                nc.gpsimd.dma_start(out=out[:, TB * t:TB * (t + 1)], in_=osb)

            def rstd_chunk(c0, c1):
                nc.vector.reciprocal(out=rstd_stage[:, c0:c1],
                                     in_=var_stage[:, c0:c1])
                nc.scalar.activation(
                    out=rstd_stage[:, c0:c1], in_=rstd_stage[:, c0:c1],
                    func=mybir.ActivationFunctionType.Sqrt)

            _st = {}
            if nt < 4:
                for t in range(nt):
                    phase1_body(t, _st)
                rstd_chunk(0, 8 * nt)
                for t in range(nt):
                    phase3_body(t)
            else:
                NCH = 4 if nt % 4 == 0 else 2
                H = nt // NCH
                for c in range(NCH):
                    for t in range(c * H, (c + 1) * H):
                        phase1_body(t, _st)
                        if c > 0:
                            phase3_body(t - H)
                    rstd_chunk(8 * c * H, 8 * (c + 1) * H)
                for t in range((NCH - 1) * H, nt):
                    phase3_body(t)
    _split_multi_waits(nc)
    return nc


def kernel(**inputs):
    inputs = {k: np.asarray(v, np.float32) for k, v in inputs.items()}
    cf, cb = _pack_consts(inputs)

    if 'nc' not in _BASS_CACHE:
        _BASS_CACHE['nc'] = _build_bass()
    nc = _BASS_CACHE['nc']

    s = np.ascontiguousarray(inputs['s_input'])
    in_maps = []
    for i in range(N_CORES):
        in_maps.append({
            "s_in": np.ascontiguousarray(s[i * BC:(i + 1) * BC].T),
            "constf": cf,
            "constb": cb,
        })
    _BASS_CACHE['in_maps'] = in_maps
    res = run_bass_kernel_spmd(nc, in_maps, core_ids=list(range(N_CORES)))
    outs = []
    for i in range(N_CORES):
        o = np.asarray(res.results[i]["out"])           # [2, BC]
        outs.append(np.ascontiguousarray(o.T))
    return np.concatenate(outs, axis=0)



# revision 18
# speedup vs baseline: 1.1465x; 1.1465x over previous
"""Trainium2 Bass kernel for nn_Actor_att1 (gnn_message_passing).

Data-parallel over 8 NeuronCores: each core processes B/8 = 32768 rows.

v2 design (vs v1 baseline at 1.556ms HW):
  - All matmuls bf16 (v1 ran L1 in fp32 = 4 cycles/row: ~800us of PE time).
  - Input host-packed to bf16 [128, BC] with row 127 = 1.0 so the L1
    stationary carries the bias in its 128th row (no bias op for h1).
  - Single-phase per-tile pipeline; no phase1/phase3 interleave.  LN rstd
    is computed tile-locally with a Newton rsqrt (bit-trick seed) on
    Pool, so the scalar engine never needs Sqrt/Lrelu -> zero ACT table
    switches (v1 paid 105 x 1283ns).  Leaky-relu is fused into DVE STT,
    MLP biases ride constant-1 rows in the stationaries.
  - LN scaling stays in transposed layout: var via per-subtile PE
    reduction to natural [128,8], Newton rsqrt, 4 tiny PE transposes back
    to [2,512], one PE replicate pass, one fused DVE (relu*rstd) op.
  - PSUM pools grouped by liveness class (producer->consumer distance) so
    tile t+1's L1/L2 overlaps tile t's attention/LN/MLP tail.
  - Elementwise balanced ACT/DVE; Pool gets SBUF-SBUF work only (no PSUM
    port on GPSIMD): score products, Newton iteration, memsets.
"""

import numpy as np
import ml_dtypes

import concourse.bass as bass
import concourse.tile as tile
from concourse import mybir
from concourse.bass_utils import run_bass_kernel_spmd

F32 = mybir.dt.float32
BF16 = mybir.dt.bfloat16
U32 = mybir.dt.uint32

N_CORES = 8
B_FULL = 262144
BC = B_FULL // N_CORES      # 32768 rows per core
TB = 512                    # batch tile
NT = BC // TB               # 64 tiles
NSUB = 4                    # 128-row subtiles per tile
EPS = 1e-5
MAGIC = 0x5f3759df          # rsqrt seed

# ---- CB (bf16) column layout ----
W1_C = 0            # 8 blocks [128,128]; row 127 = b1 bias row
W2_C = 1024         # 8 blocks [128,128]
SCORE_C = 2048      # 4 blocks [128,32]
REPW_C = 2176       # 4 blocks [32,128]
REPC_C = 2688       # 4 blocks [128,48]
WSELF_C = 2880      # [32,128]: en_w2 replicated 8x along cols
SQONES_C = 3008     # [48,2]
EYEB_C = 3010       # [128,128]
M1_C = 3138         # [49,32]
M2_C = 3170         # [33,32]
M3_C = 3202         # [33,2]
REP3_C = 3204       # [3,48]
BF_COLS = 3252

# ---- CF (f32) column layout ----
B2SB_C = 0          # 4 cols [128,1]: E bias per psum half
MAGIC_C = 4         # rsqrt magic constant (u32 bit pattern)
B2SELF_C = 5        # [128,1]: en_b2 replicated 8x
EYE_C = 8           # [128,128]
F_COLS = 136

_BASS_CACHE = {}


def _pack_consts(p):
    cb = np.zeros((128, BF_COLS), np.float32)
    cf = np.zeros((128, F_COLS), np.float32)

    # --- W1 block-diag [127, 1024] + bias row ---
    w1 = np.zeros((128, 1024), np.float32)
    w1[0:4, 0:32] = p['en_w1']
    w1[127, 0:32] = p['en_b1']
    for i in range(15):
        c = 32 + 32 * i
        w1[4 + 2 * i, c:c + 32] = p['oa_w1'][0]
        w1[5 + 2 * i, c:c + 32] = p['oa_w1'][1]
        w1[34 + 2 * i, c:c + 32] = p['oa_w1'][2]
        w1[35 + 2 * i, c:c + 32] = p['oa_w1'][3]
        w1[64 + i, c:c + 32] = p['oa_w1'][4]
        w1[127, c:c + 32] = p['oa_b1']
    for j in range(16):
        c = 512 + 32 * j
        for k in range(3):
            w1[79 + 3 * j + k, c:c + 32] = p['g_w1'][k]
        w1[127, c:c + 32] = p['g_b1']
    cb[:, W1_C:W1_C + 1024] = w1

    # --- W2 block-diag: agent a -> h1 block g=a//4, psum half jj=a//8 ---
    w2s = [p['en_w2']] + [p['oa_w2']] * 15 + [p['g_w2']] * 16
    b2s = [p['en_b2']] + [p['oa_b2']] * 15 + [p['g_b2']] * 16
    w2big = np.zeros((128, 1024), np.float32)
    b2big = np.zeros(512, np.float32)
    for a in range(32):
        g, jj = a // 4, a // 8
        al = a % 4
        w2big[32 * al:32 * al + 32,
              128 * g + 16 * (a - 8 * jj):128 * g + 16 * (a - 8 * jj) + 16] = w2s[a]
        b2big[16 * a:16 * a + 16] = b2s[a]
    cb[:, W2_C:W2_C + 1024] = w2big
    cf[:, B2SB_C:B2SB_C + 4] = b2big.reshape(4, 128).T
    cf[:, MAGIC_C] = np.frombuffer(
        np.full(1, MAGIC, np.uint32).tobytes(), np.float32)[0]
    cf[:, B2SELF_C] = np.tile(p['en_b2'], 8)
    cf[0:128, EYE_C:EYE_C + 128] = np.eye(128, dtype=np.float32)

    # --- attention matrices per feature-block j (agents 8j..8j+7) ---
    for j in range(4):
        so = np.zeros((128, 32), np.float32)
        rc = np.zeros((128, 48), np.float32)
        rw = np.zeros((32, 128), np.float32)
        for nl in range(8):
            a = 8 * j + nl
            if a == 0:
                continue
            col = (a - 1) if a < 16 else (16 + a - 16)
            t = 0 if a < 16 else 1
            so[16 * nl:16 * nl + 16, col] = 1.0
            rw[col, 16 * nl:16 * nl + 16] = 1.0
            blk = np.eye(16, dtype=np.float32) - 1.0 / 16.0
            rc[16 * nl:16 * nl + 16, 16 * t:16 * t + 16] = blk
        if j == 0:
            rc[np.arange(16), 32 + np.arange(16)] = 1.0   # self passthrough
            rw[15, 0:16] = 1.0    # score row 15 unused -> exp(0)=1 -> self
        cb[:, SCORE_C + 32 * j:SCORE_C + 32 * j + 32] = so
        cb[:, REPC_C + 48 * j:REPC_C + 48 * j + 48] = rc
        cb[0:32, REPW_C + 128 * j:REPW_C + 128 * j + 128] = rw
    wself = np.zeros((32, 128), np.float32)
    for k in range(8):
        wself[:, 16 * k:16 * k + 16] = p['en_w2']
    cb[0:32, WSELF_C:WSELF_C + 128] = wself
    sq = np.zeros((48, 2), np.float32)
    sq[0:16, 0] = 1.0 / 16.0
    sq[16:32, 1] = 1.0 / 16.0
    cb[0:48, SQONES_C:SQONES_C + 2] = sq
    cb[:, EYEB_C:EYEB_C + 128] = np.eye(128, dtype=np.float32)

    # --- final MLP with bias rows; msb rows = [other, food, self, ones] ---
    m_w1 = p['m_w1']  # [48,32], merged order [self, food, other]
    cb[0:16, M1_C:M1_C + 32] = m_w1[32:48]
    cb[16:32, M1_C:M1_C + 32] = m_w1[16:32]
    cb[32:48, M1_C:M1_C + 32] = m_w1[0:16]
    cb[48, M1_C:M1_C + 32] = p['m_b1']
    cb[0:32, M2_C:M2_C + 32] = p['m_w2']
    cb[32, M2_C:M2_C + 32] = p['m_b2']
    cb[0:32, M3_C:M3_C + 2] = p['m_w3']
    cb[32, M3_C:M3_C + 2] = p['m_b3']

    # rstd replicate: C rows [other|food|self] x [rstd0|rstd1|1.0]
    rep3 = np.zeros((3, 48), np.float32)
    rep3[0, 0:16] = 1.0
    rep3[1, 16:32] = 1.0
    rep3[2, 32:48] = 1.0
    cb[0:3, REP3_C:REP3_C + 48] = rep3

    for k in ('oa_g', 'g_g'):
        assert np.allclose(p[k], 1.0), "LN gain != 1 unsupported"
    for k in ('oa_bln', 'g_bln'):
        assert np.allclose(p[k], 0.0), "LN bias != 0 unsupported"

    return cb.astype(ml_dtypes.bfloat16), cf


def _split_multi_waits(nc):
    """Walrus accepts only one sync-wait per instruction; move extra waits
    onto dedicated EventSemaphore instructions just before."""
    f = nc.m.functions[0]
    ctr = 0
    for blk in f.blocks:
        new_ins = []
        for ins in blk.instructions:
            si = getattr(ins, 'sync_info', None)
            ow = list(si.on_wait) if si is not None and si.on_wait else []
            if len(ow) > 1:
                for w in ow[:-1]:
                    ev = mybir.InstEventSemaphore(
                        name=f"wsplit_{ctr}",
                        engine=ins.engine,
                        ins=[], outs=[],
                        sync_info=mybir.SyncInfo(on_wait=[w], on_update=[]),
                    )
                    ctr += 1
                    new_ins.append(ev)
                si.on_wait = ow[-1:]
            new_ins.append(ins)
        blk.instructions[:] = new_ins
    return ctr


def _build_bass(nt=NT, split_waits=True, dbg=False, prelu='act'):
    nc = bass.Bass()
    s_in = nc.dram_tensor("s_in", [128, BC], BF16, kind="ExternalInput")
    cbd = nc.dram_tensor("cb", [128, BF_COLS], BF16, kind="ExternalInput")
    cfd = nc.dram_tensor("cf", [128, F_COLS], F32, kind="ExternalInput")
    out = nc.dram_tensor("out", [2, NT * TB], F32, kind="ExternalOutput")
    dbgs = {}
    if dbg:
        for nm, shp, dt in [("d_h10", [128, TB], BF16), ("d_E0", [128, TB], BF16),
                            ("d_sr", [128, TB], BF16),
                            ("d_wt", [32, TB], BF16), ("d_sqb", [48, TB], BF16),
                            ("d_ve", [128, 8], F32), ("d_rstdn", [128, 8], BF16),
                            ("d_msb", [49, TB], BF16),
                            ("d_hh1", [33, TB], BF16)]:
            dbgs[nm] = nc.dram_tensor(nm, shp, dt, kind="ExternalOutput")

    def dump(nm, ap):
        if dbg:
            nc.gpsimd.dma_start(out=dbgs[nm][:, :], in_=ap)

    AF = mybir.ActivationFunctionType
    AL = mybir.AluOpType

    with tile.TileContext(nc) as tc:
        with (
            tc.tile_pool(name="singles", bufs=1) as singles,
            tc.tile_pool(name="xt", bufs=2) as xt_p,
            tc.tile_pool(name="h1", bufs=2) as h1_p,
            tc.tile_pool(name="enc", bufs=2) as enc_p,
            tc.tile_pool(name="work", bufs=3) as work_p,
            tc.tile_pool(name="phe", bufs=3, space="PSUM") as phe,    # 3 banks
            tc.tile_pool(name="pw", bufs=3, space="PSUM") as pw,      # 3 banks
            tc.tile_pool(name="pc", bufs=1, space="PSUM") as pc,      # 1 bank
            tc.tile_pool(name="pv", bufs=1, space="PSUM") as pv,      # 1 bank
        ):
            CB = singles.tile([128, BF_COLS], BF16)
            CF = singles.tile([128, F_COLS], F32)
            nc.sync.dma_start(out=CB, in_=cbd[:, :])
            nc.sync.dma_start(out=CF, in_=cfd[:, :])
            eyeb = CB[:, EYEB_C:EYEB_C + 128]
            eye = CF[:, EYE_C:EYE_C + 128]

            # warm-up: every engine observes both const DMAs once so later
            # instructions carry at most one fresh sync-wait each.
            scratch = singles.tile([1, 48], F32)
            wf = pw.tile([128, 128], F32, tag="w")
            nc.tensor.transpose(wf[0:128, 0:128], eye, eye)
            nc.vector.tensor_copy(out=scratch[0:1, 0:8], in_=wf[0:1, 0:8])
            wb = pw.tile([128, 128], BF16, tag="w")
            nc.tensor.transpose(wb[0:128, 0:128], eyeb, eyeb)
            nc.vector.tensor_copy(out=scratch[0:1, 8:16], in_=wb[0:1, 0:8])
            nc.scalar.copy(out=scratch[0:1, 16:24], in_=CF[0:1, 0:8])
            nc.scalar.copy(out=scratch[0:1, 24:32], in_=CB[0:1, 0:8])
            nc.vector.tensor_copy(out=scratch[0:1, 32:40], in_=CF[0:1, 0:8])
            nc.vector.tensor_copy(out=scratch[0:1, 40:48], in_=CB[0:1, 0:8])
            gs = singles.tile([1, 16], BF16)
            nc.gpsimd.tensor_copy(out=gs[0:1, 0:8], in_=CB[0:1, 0:8])

            _st = {}

            def tile_body(t):
                r0 = t * TB
                if t % 2 == 0:
                    xT2 = xt_p.tile([128, 2 * TB], BF16, tag="xT")
                    _st['xT2'] = xT2
                    # absorber: Pool observes the WAR tick so the DMA itself
                    # carries only its lane wait
                    nc.gpsimd.memset(xT2[0:1, 0:4], 0.0)
                    nc.gpsimd.dma_start(out=xT2, in_=s_in[:, r0:r0 + 2 * TB])
                    xT = xT2[:, 0:TB]
                else:
                    xT = _st['xT2'][:, TB:2 * TB]

                # ---- L1: 8 single psums; bias via ones row; pure-relu ----
                h1t = []
                for g in range(8):
                    hp = phe.tile([128, TB], F32, tag="he")
                    nc.tensor.matmul(
                        hp, CB[:, W1_C + 128 * g:W1_C + 128 * (g + 1)],
                        xT, start=True, stop=True)
                    hg = h1_p.tile([128, TB], BF16, tag=f"h1{g}")
                    if g < 5:
                        nc.scalar.activation(out=hg, in_=hp, func=AF.Relu)
                    else:
                        nc.vector.tensor_scalar(
                            out=hg, in0=hp, scalar1=0.0, scalar2=None,
                            op0=AL.max)
                    h1t.append(hg)
                if t == 0:
                    dump("d_h10", h1t[0][:, :])

                # ---- self enc replicated: W2SELF @ h1[0] ----
                srp = pw.tile([128, TB], F32, tag="w")
                nc.tensor.matmul(srp, CB[0:32, WSELF_C:WSELF_C + 128],
                                 h1t[0][0:32, :], start=True, stop=True)
                sr = work_p.tile([128, TB], BF16, tag="sr")
                nc.vector.tensor_scalar(
                    out=sr, in0=srp, scalar1=CF[:, B2SELF_C:B2SELF_C + 1],
                    scalar2=0.0, op0=AL.add, op1=AL.max)
                if t == 0:
                    dump("d_sr", sr[:, :])

                # ---- L2: 4 single psums; E relu+bias into pair tiles ----
                E0t = enc_p.tile([128, 2 * TB], BF16, tag="E0")
                E1t = enc_p.tile([128, 2 * TB], BF16, tag="E1")
                Epr = [E0t, E1t]
                for jj in range(4):
                    ep = phe.tile([128, TB], F32, tag="he")
                    for k in range(2):
                        g = 2 * jj + k
                        nc.tensor.matmul(
                            ep, CB[:, W2_C + 128 * g:W2_C + 128 * (g + 1)],
                            h1t[g], start=(k == 0), stop=(k == 1))
                    dst = Epr[jj // 2][:, TB * (jj % 2):TB * (jj % 2 + 1)]
                    bias = CF[:, B2SB_C + jj:B2SB_C + jj + 1]
                    if jj < 2:
                        nc.scalar.activation(out=dst, in_=ep, func=AF.Relu,
                                             bias=bias, scale=1.0)
                    else:
                        nc.vector.tensor_scalar(
                            out=dst, in0=ep, scalar1=bias, scalar2=0.0,
                            op0=AL.add, op1=AL.max)
                if t == 0:
                    dump("d_E0", Epr[0][:, 0:TB])

                # ---- scores: P = E*sr (Pool), 4 accumulating matmuls ----
                P_pr = []
                srs = sr[:, :]
                srb = bass.AP(tensor=srs.tensor, offset=srs.offset,
                              ap=[srs.ap[0], [0, 2], srs.ap[1]])
                for e in range(2):
                    pjp = work_p.tile([128, 2 * TB], BF16, tag=f"P{e}")
                    nc.gpsimd.tensor_mul(
                        pjp.rearrange("p (k n) -> p k n", k=2),
                        Epr[e].rearrange("p (k n) -> p k n", k=2), srb)
                    P_pr.append(pjp)
                S = pw.tile([32, TB], F32, tag="w")
                for jj in range(4):
                    nc.tensor.matmul(
                        S, CB[:, SCORE_C + 32 * jj:SCORE_C + 32 * (jj + 1)],
                        P_pr[jj // 2][:, TB * (jj % 2):TB * (jj % 2 + 1)],
                        start=(jj == 0), stop=(jj == 3))

                # ---- wt = exp(score/4); wr = replicate; P2 = E*wr ----
                wt = work_p.tile([32, TB], BF16, tag="wt")
                nc.scalar.activation(out=wt, in_=S, func=AF.Exp, scale=0.25)
                if t == 0:
                    dump("d_wt", wt[:, :])
                P2_pr = []
                for e in range(2):
                    p2 = work_p.tile([128, 2 * TB], BF16, tag=f"P2{e}")
                    for h in range(2):
                        jj = 2 * e + h
                        wrp = pw.tile([128, TB], F32, tag="w")
                        nc.tensor.matmul(
                            wrp,
                            CB[0:32, REPW_C + 128 * jj:REPW_C + 128 * (jj + 1)],
                            wt, start=True, stop=True)
                        nc.vector.tensor_mul(
                            p2[:, TB * h:TB * (h + 1)],
                            Epr[e][:, TB * h:TB * (h + 1)], wrp)
                    P2_pr.append(p2)

                # ---- C: centered numerators + self passthrough ----
                C = pc.tile([48, TB], F32, tag="c")
                for jj in range(4):
                    nc.tensor.matmul(
                        C, CB[:, REPC_C + 48 * jj:REPC_C + 48 * (jj + 1)],
                        P2_pr[jj // 2][:, TB * (jj % 2):TB * (jj % 2 + 1)],
                        start=(jj == 0), stop=(jj == 3))

                # ---- variance (natural) + Newton rsqrt (Pool) ----
                sqb = work_p.tile([48, TB], BF16, tag="sqb")
                nc.scalar.activation(out=sqb, in_=C, func=AF.Square)
                if t == 0:
                    dump("d_sqb", sqb[:, :])
                vn = pv.tile([128, 8], F32, tag="v")
                for s in range(NSUB):
                    nc.tensor.matmul(
                        vn[:, 2 * s:2 * s + 2],
                        sqb[:, 128 * s:128 * (s + 1)],
                        CB[0:48, SQONES_C:SQONES_C + 2],
                        start=True, stop=True)
                ve = work_p.tile([128, 8], F32, tag="ve")
                nc.vector.tensor_scalar(out=ve, in0=vn, scalar1=EPS,
                                        scalar2=None, op0=AL.add)
                veu = ve.bitcast(U32)
                y0b = work_p.tile([128, 8], U32, tag="y0b")
                nc.vector.tensor_scalar(out=y0b, in0=veu, scalar1=1,
                                        scalar2=None,
                                        op0=AL.logical_shift_right)
                mgs = CF[:, MAGIC_C:MAGIC_C + 1].bitcast(U32)
                mgb = bass.AP(tensor=mgs.tensor, offset=mgs.offset,
                              ap=[mgs.ap[0], [0, 8]])
                nc.gpsimd.tensor_sub(out=y0b, in0=mgb, in1=y0b)
                y0 = y0b.bitcast(F32)
                tt = work_p.tile([128, 8], F32, tag="tt")
                nc.gpsimd.tensor_tensor(out=tt, in0=ve, in1=y0, op=AL.mult)
                nc.gpsimd.tensor_tensor(out=tt, in0=tt, in1=y0, op=AL.mult)
                nc.vector.tensor_scalar(out=tt, in0=tt, scalar1=-0.5,
                                        scalar2=1.5, op0=AL.mult, op1=AL.add)
                rstdn = work_p.tile([128, 8], BF16, tag="rstdn")
                nc.gpsimd.tensor_tensor(out=rstdn, in0=y0, in1=tt, op=AL.mult)
                if t == 0:
                    dump("d_ve", ve[:, :])
                    dump("d_rstdn", rstdn[:, :])

                # transpose rstd to [2,512]; ones row; replicate to [48,512]
                rT = pv.tile([2, TB], BF16, tag="v")
                for s in range(NSUB):
                    nc.tensor.transpose(
                        rT[:, 128 * s:128 * (s + 1)],
                        rstdn[:, 2 * s:2 * s + 2], eyeb)
                rstdt = work_p.tile([3, TB], BF16, tag="rstdt")
                nc.gpsimd.memset(rstdt, 1.0)
                nc.vector.tensor_copy(out=rstdt[0:2, :], in_=rT)
                rrep = pv.tile([48, TB], F32, tag="v")
                nc.tensor.matmul(rrep, CB[0:3, REP3_C:REP3_C + 48], rstdt,
                                 start=True, stop=True)

                # ---- mn = relu(C) * rstd_rep; ones row for bias ----
                rn = work_p.tile([48, TB], BF16, tag="rn")
                nc.scalar.activation(out=rn, in_=C, func=AF.Relu)
                msb = work_p.tile([49, TB], BF16, tag="msb")
                nc.gpsimd.memset(msb, 1.0)
                nc.vector.tensor_tensor(out=msb[0:48, :], in0=rn, in1=rrep,
                                        op=AL.mult)
                if t == 0:
                    dump("d_msb", msb[:, :])

                # ---- final MLP, leaky-relu via DVE STT ----
                h1f = pw.tile([32, TB], F32, tag="w")
                nc.tensor.matmul(h1f, CB[0:49, M1_C:M1_C + 32], msb,
                                 start=True, stop=True)
                hh1 = work_p.tile([33, TB], BF16, tag="hh1")
                nc.gpsimd.memset(hh1, 1.0)
                if prelu == 'dve':
                    lt1 = work_p.tile([32, TB], BF16, tag="lt1")
                    nc.vector.tensor_scalar(out=lt1, in0=h1f, scalar1=0.01,
                                            scalar2=None, op0=AL.mult)
                    nc.vector.tensor_tensor(out=hh1[0:32, :], in0=lt1,
                                            in1=h1f, op=AL.max)
                else:
                    nc.scalar.activation(
                        out=hh1[0:32, :], in_=h1f,
                        func=AF.Prelu if prelu == 'act' else AF.Lrelu,
                        alpha=0.01)
                if t == 0:
                    dump("d_hh1", hh1[:, :])
                h2f = pw.tile([32, TB], F32, tag="w")
                nc.tensor.matmul(h2f, CB[0:33, M2_C:M2_C + 32], hh1,
                                 start=True, stop=True)
                hh2 = work_p.tile([33, TB], BF16, tag="hh2")
                nc.gpsimd.memset(hh2, 1.0)
                if prelu == 'dve':
                    lt2 = work_p.tile([32, TB], BF16, tag="lt2")
                    nc.vector.tensor_scalar(out=lt2, in0=h2f, scalar1=0.01,
                                            scalar2=None, op0=AL.mult)
                    nc.vector.tensor_tensor(out=hh2[0:32, :], in0=lt2,
                                            in1=h2f, op=AL.max)
                else:
                    nc.scalar.activation(
                        out=hh2[0:32, :], in_=h2f,
                        func=AF.Prelu if prelu == 'act' else AF.Lrelu,
                        alpha=0.01)
                of = pw.tile([2, TB], F32, tag="w")
                nc.tensor.matmul(of, CB[0:33, M3_C:M3_C + 2], hh2,
                                 start=True, stop=True)
                osb = work_p.tile([2, TB], F32, tag="osb")
                nc.scalar.activation(out=osb, in_=of, func=AF.Tanh)

                nc.gpsimd.tensor_copy(out=gs[0:1, 8:12],
                                      in_=osb[0:1, 508:512])
                nc.gpsimd.dma_start(out=out[:, TB * t:TB * (t + 1)], in_=osb)

            for t in range(nt):
                tile_body(t)

    if split_waits:
        _split_multi_waits(nc)
    return nc


def kernel(**inputs):
    inputs = {k: np.asarray(v, np.float32) for k, v in inputs.items()}
    cb, cf = _pack_consts(inputs)

    if 'nc' not in _BASS_CACHE:
        _BASS_CACHE['nc'] = _build_bass()
    nc = _BASS_CACHE['nc']

    s = np.ascontiguousarray(inputs['s_input'])
    in_maps = []
    for i in range(N_CORES):
        xt = np.ones((128, BC), np.float32)
        xt[0:127] = s[i * BC:(i + 1) * BC].T
        in_maps.append({
            "s_in": xt.astype(ml_dtypes.bfloat16),
            "cb": cb,
            "cf": cf,
        })
    _BASS_CACHE['in_maps'] = in_maps
    res = run_bass_kernel_spmd(nc, in_maps, core_ids=list(range(N_CORES)))
    outs = []
    for i in range(N_CORES):
        o = np.asarray(res.results[i]["out"])           # [2, BC]
        outs.append(np.ascontiguousarray(o.T))
    return np.concatenate(outs, axis=0)


# revision 19
# speedup vs baseline: 1.4882x; 1.2980x over previous
"""Trainium2 Bass kernel for nn_Actor_att1 (gnn_message_passing).

Data-parallel over 8 NeuronCores: each core processes B/8 = 32768 rows.

v2 design (vs v1 baseline at 1.556ms HW):
  - All matmuls bf16 (v1 ran L1 in fp32 = 4 cycles/row: ~800us of PE time).
  - Input host-packed to bf16 [128, BC] with row 127 = 1.0 so the L1
    stationary carries the bias in its 128th row (no bias op for h1).
  - Single-phase per-tile pipeline; no phase1/phase3 interleave.  LN rstd
    is computed tile-locally with a Newton rsqrt (bit-trick seed) on
    Pool, so the scalar engine never needs Sqrt/Lrelu -> zero ACT table
    switches (v1 paid 105 x 1283ns).  Leaky-relu is fused into DVE STT,
    MLP biases ride constant-1 rows in the stationaries.
  - LN scaling stays in transposed layout: var via per-subtile PE
    reduction to natural [128,8], Newton rsqrt, 4 tiny PE transposes back
    to [2,512], one PE replicate pass, one fused DVE (relu*rstd) op.
  - PSUM pools grouped by liveness class (producer->consumer distance) so
    tile t+1's L1/L2 overlaps tile t's attention/LN/MLP tail.
  - Elementwise balanced ACT/DVE; Pool gets SBUF-SBUF work only (no PSUM
    port on GPSIMD): score products, Newton iteration, memsets.
"""

import numpy as np
import ml_dtypes

import concourse.bass as bass
import concourse.tile as tile
from concourse import mybir
from concourse.bass_utils import run_bass_kernel_spmd

F32 = mybir.dt.float32
BF16 = mybir.dt.bfloat16
U32 = mybir.dt.uint32

N_CORES = 8
B_FULL = 262144
BC = B_FULL // N_CORES      # 32768 rows per core
TB = 512                    # batch tile
NT = BC // TB               # 64 tiles
NSUB = 4                    # 128-row subtiles per tile
EPS = 1e-5
MAGIC = 0x5f3759df          # rsqrt seed

# ---- CB (bf16) column layout ----
W1_C = 0            # 8 blocks [128,128]; row 127 = b1 bias row
W2_C = 1024         # 8 blocks [128,128]
SCORE_C = 2048      # 4 blocks [128,32]
REPC_C = 2176       # 4 blocks [128,64] (rows 48:64 of out zeroed)
WSELF_C = 2432      # [32,128]: en_w2 replicated 8x along cols
SQONES_C = 2560     # [48,2] at rows 0:48 AND rows 64:112
EYEB_C = 2562       # [128,128]
REP3_C = 2690       # [3,64]
REPW4_C = 2754      # 8 blocks [32@rows 32k, 128]: (k,j)
M1_C = 3778         # 2 copies [48,32] at rows 0 / 64
M2_C = 3842         # diag [32@32k, 32] blocks x2
M3_C = 3906         # [32@32k, 32] blocks x2 (cols 2:32 zero)
BF_COLS = 3970

# ---- CF (f32) column layout ----
B2SB_C = 0          # 4 cols [128,1]: E bias per psum half
MAGIC_C = 4         # rsqrt magic constant (u32 bit pattern)
B2SELF_C = 5        # [128,1]: en_b2 replicated 8x
B1T_C = 6           # m_b1 tiled at rows 32k+i
B2T_C = 7           # m_b2 tiled
EYE_C = 8           # [128,128]
B3T_C = 136         # m_b3 at rows {32k, 32k+1}
F_COLS = 140

_BASS_CACHE = {}


def _pack_consts(p):
    cb = np.zeros((128, BF_COLS), np.float32)
    cf = np.zeros((128, F_COLS), np.float32)

    # --- W1 block-diag [127, 1024] + bias row ---
    w1 = np.zeros((128, 1024), np.float32)
    w1[0:4, 0:32] = p['en_w1']
    w1[127, 0:32] = p['en_b1']
    for i in range(15):
        c = 32 + 32 * i
        w1[4 + 2 * i, c:c + 32] = p['oa_w1'][0]
        w1[5 + 2 * i, c:c + 32] = p['oa_w1'][1]
        w1[34 + 2 * i, c:c + 32] = p['oa_w1'][2]
        w1[35 + 2 * i, c:c + 32] = p['oa_w1'][3]
        w1[64 + i, c:c + 32] = p['oa_w1'][4]
        w1[127, c:c + 32] = p['oa_b1']
    for j in range(16):
        c = 512 + 32 * j
        for k in range(3):
            w1[79 + 3 * j + k, c:c + 32] = p['g_w1'][k]
        w1[127, c:c + 32] = p['g_b1']
    cb[:, W1_C:W1_C + 1024] = w1

    # --- W2 block-diag: agent a -> h1 block g=a//4, psum half jj=a//8 ---
    w2s = [p['en_w2']] + [p['oa_w2']] * 15 + [p['g_w2']] * 16
    b2s = [p['en_b2']] + [p['oa_b2']] * 15 + [p['g_b2']] * 16
    w2big = np.zeros((128, 1024), np.float32)
    b2big = np.zeros(512, np.float32)
    for a in range(32):
        g, jj = a // 4, a // 8
        al = a % 4
        w2big[32 * al:32 * al + 32,
              128 * g + 16 * (a - 8 * jj):128 * g + 16 * (a - 8 * jj) + 16] = w2s[a]
        b2big[16 * a:16 * a + 16] = b2s[a]
    cb[:, W2_C:W2_C + 1024] = w2big
    cf[:, B2SB_C:B2SB_C + 4] = b2big.reshape(4, 128).T
    cf[:, MAGIC_C] = np.frombuffer(
        np.full(1, MAGIC, np.uint32).tobytes(), np.float32)[0]
    cf[:, B2SELF_C] = np.tile(p['en_b2'], 8)
    cf[0:128, EYE_C:EYE_C + 128] = np.eye(128, dtype=np.float32)

    # --- attention matrices per feature-block j (agents 8j..8j+7) ---
    for j in range(4):
        so = np.zeros((128, 32), np.float32)
        rc = np.zeros((128, 64), np.float32)
        rw = np.zeros((32, 128), np.float32)
        for nl in range(8):
            a = 8 * j + nl
            if a == 0:
                continue
            col = (a - 1) if a < 16 else (16 + a - 16)
            t = 0 if a < 16 else 1
            so[16 * nl:16 * nl + 16, col] = 1.0
            rw[col, 16 * nl:16 * nl + 16] = 1.0
            blk = np.eye(16, dtype=np.float32) - 1.0 / 16.0
            rc[16 * nl:16 * nl + 16, 16 * t:16 * t + 16] = blk
        if j == 0:
            rc[np.arange(16), 32 + np.arange(16)] = 1.0   # self passthrough
            rw[15, 0:16] = 1.0    # score row 15 unused -> exp(0)=1 -> self
        cb[:, SCORE_C + 32 * j:SCORE_C + 32 * j + 32] = so
        cb[:, REPC_C + 64 * j:REPC_C + 64 * j + 64] = rc
        for k in range(2):
            c0 = REPW4_C + 128 * (4 * k + j)
            cb[32 * k:32 * k + 32, c0:c0 + 128] = rw
    wself = np.zeros((32, 128), np.float32)
    for k in range(8):
        wself[:, 16 * k:16 * k + 16] = p['en_w2']
    cb[0:32, WSELF_C:WSELF_C + 128] = wself
    sq = np.zeros((48, 2), np.float32)
    sq[0:16, 0] = 1.0 / 16.0
    sq[16:32, 1] = 1.0 / 16.0
    cb[0:48, SQONES_C:SQONES_C + 2] = sq
    cb[64:112, SQONES_C:SQONES_C + 2] = sq
    cb[:, EYEB_C:EYEB_C + 128] = np.eye(128, dtype=np.float32)

    # --- final MLP; biases via ACT bias columns, quad-stacked layouts ---
    m_w1 = p['m_w1']  # [48,32], merged order [self, food, other]
    m1r = np.concatenate([m_w1[32:48], m_w1[16:32], m_w1[0:16]], 0)
    cb[0:48, M1_C:M1_C + 32] = m1r
    cb[64:112, M1_C + 32:M1_C + 64] = m1r
    for k in range(2):
        cb[32 * k:32 * k + 32, M2_C + 32 * k:M2_C + 32 * k + 32] = p['m_w2']
        cb[32 * k:32 * k + 32, M3_C + 32 * k:M3_C + 32 * k + 2] = p['m_w3']
        cf[32 * k:32 * k + 32, B1T_C] = p['m_b1']
        cf[32 * k:32 * k + 32, B2T_C] = p['m_b2']
        cf[32 * k:32 * k + 2, B3T_C] = p['m_b3']

    # rstd replicate: C rows [other|food|self] x [rstd0|rstd1|1.0]
    rep3 = np.zeros((3, 64), np.float32)
    rep3[0, 0:16] = 1.0
    rep3[1, 16:32] = 1.0
    rep3[2, 32:48] = 1.0
    cb[0:3, REP3_C:REP3_C + 64] = rep3

    for k in ('oa_g', 'g_g'):
        assert np.allclose(p[k], 1.0), "LN gain != 1 unsupported"
    for k in ('oa_bln', 'g_bln'):
        assert np.allclose(p[k], 0.0), "LN bias != 0 unsupported"

    return cb.astype(ml_dtypes.bfloat16), cf


def _split_multi_waits(nc):
    """Walrus accepts only one sync-wait per instruction; move extra waits
    onto dedicated EventSemaphore instructions just before."""
    f = nc.m.functions[0]
    ctr = 0
    for blk in f.blocks:
        new_ins = []
        for ins in blk.instructions:
            si = getattr(ins, 'sync_info', None)
            ow = list(si.on_wait) if si is not None and si.on_wait else []
            if len(ow) > 1:
                for w in ow[:-1]:
                    ev = mybir.InstEventSemaphore(
                        name=f"wsplit_{ctr}",
                        engine=ins.engine,
                        ins=[], outs=[],
                        sync_info=mybir.SyncInfo(on_wait=[w], on_update=[]),
                    )
                    ctr += 1
                    new_ins.append(ev)
                si.on_wait = ow[-1:]
            new_ins.append(ins)
        blk.instructions[:] = new_ins
    return ctr


def _build_bass(nt=NT, split_waits=True, dbg=False, prelu='act'):
    nc = bass.Bass()
    s_in = nc.dram_tensor("s_in", [128, BC], BF16, kind="ExternalInput")
    cbd = nc.dram_tensor("cb", [128, BF_COLS], BF16, kind="ExternalInput")
    cfd = nc.dram_tensor("cf", [128, F_COLS], F32, kind="ExternalInput")
    out = nc.dram_tensor("out", [2, NT * TB], F32, kind="ExternalOutput")
    dbgs = {}
    if dbg:
        for nm, shp, dt in [("d_h10", [128, TB], BF16), ("d_E0", [128, TB], BF16),
                            ("d_sr", [128, TB], BF16),
                            ("d_wt", [32, TB], BF16), ("d_sqb", [48, TB], BF16),
                            ("d_ve", [128, 8], F32), ("d_rstdn", [128, 8], BF16),
                            ("d_msb", [49, TB], BF16),
                            ("d_hh1", [33, TB], BF16)]:
            dbgs[nm] = nc.dram_tensor(nm, shp, dt, kind="ExternalOutput")

    def dump(nm, ap):
        if dbg:
            nc.gpsimd.dma_start(out=dbgs[nm][:, :], in_=ap)

    AF = mybir.ActivationFunctionType
    AL = mybir.AluOpType

    with tile.TileContext(nc) as tc:
        with (
            tc.tile_pool(name="singles", bufs=1) as singles,
            tc.tile_pool(name="xt", bufs=2) as xt_p,
            tc.tile_pool(name="h1", bufs=2) as h1_p,
            tc.tile_pool(name="enc", bufs=2) as enc_p,
            tc.tile_pool(name="work", bufs=3) as work_p,
            tc.tile_pool(name="phe", bufs=3, space="PSUM") as phe,    # 3 banks
            tc.tile_pool(name="pw", bufs=2, space="PSUM") as pw,      # 2 banks
            tc.tile_pool(name="pq", bufs=2, space="PSUM") as pq,      # 2 banks
            tc.tile_pool(name="pcv", bufs=1, space="PSUM") as pcv,    # 1 bank
        ):
            CB = singles.tile([128, BF_COLS], BF16)
            CF = singles.tile([128, F_COLS], F32)
            nc.sync.dma_start(out=CB, in_=cbd[:, :])
            nc.sync.dma_start(out=CF, in_=cfd[:, :])
            eyeb = CB[:, EYEB_C:EYEB_C + 128]
            eye = CF[:, EYE_C:EYE_C + 128]

            # warm-up: every engine observes both const DMAs once so later
            # instructions carry at most one fresh sync-wait each.
            scratch = singles.tile([1, 48], F32)
            wf = pw.tile([128, 128], F32, tag="w")
            nc.tensor.transpose(wf[0:128, 0:128], eye, eye)
            nc.vector.tensor_copy(out=scratch[0:1, 0:8], in_=wf[0:1, 0:8])
            wb = pw.tile([128, 128], BF16, tag="w")
            nc.tensor.transpose(wb[0:128, 0:128], eyeb, eyeb)
            nc.vector.tensor_copy(out=scratch[0:1, 8:16], in_=wb[0:1, 0:8])
            nc.scalar.copy(out=scratch[0:1, 16:24], in_=CF[0:1, 0:8])
            nc.scalar.copy(out=scratch[0:1, 24:32], in_=CB[0:1, 0:8])
            nc.vector.tensor_copy(out=scratch[0:1, 32:40], in_=CF[0:1, 0:8])
            nc.vector.tensor_copy(out=scratch[0:1, 40:48], in_=CB[0:1, 0:8])
            gs = singles.tile([1, 16], BF16)
            nc.gpsimd.tensor_copy(out=gs[0:1, 0:8], in_=CB[0:1, 0:8])

            _st = {}

            def stage_a(t):
                k = t % 4
                r0 = t * TB
                if t % 2 == 0:
                    xT2 = xt_p.tile([128, 2 * TB], BF16, tag="xT")
                    _st['xT2'] = xT2
                    nc.gpsimd.memset(xT2[0:1, 0:4], 0.0)
                    nc.gpsimd.dma_start(out=xT2, in_=s_in[:, r0:r0 + 2 * TB])
                    xT = xT2[:, 0:TB]
                else:
                    xT = _st['xT2'][:, TB:2 * TB]

                # L1: 8 single psums; bias via ones row; pure-relu TS
                h1t = []
                for g in range(8):
                    hp = phe.tile([128, TB], F32, tag="he")
                    nc.tensor.matmul(
                        hp, CB[:, W1_C + 128 * g:W1_C + 128 * (g + 1)],
                        xT, start=True, stop=True)
                    hg = h1_p.tile([128, TB], BF16, tag=f"h1{g}")
                    if g < 4:
                        nc.scalar.activation(out=hg, in_=hp, func=AF.Relu)
                    else:
                        nc.vector.tensor_scalar(
                            out=hg, in0=hp, scalar1=0.0, scalar2=None,
                            op0=AL.max)
                    h1t.append(hg)

                # self enc replicated: W2SELF @ h1[0]
                srp = pw.tile([128, TB], F32, tag="w")
                nc.tensor.matmul(srp, CB[0:32, WSELF_C:WSELF_C + 128],
                                 h1t[0][0:32, :], start=True, stop=True)
                sr = work_p.tile([128, TB], BF16, tag="sr")
                nc.vector.tensor_scalar(
                    out=sr, in0=srp, scalar1=CF[:, B2SELF_C:B2SELF_C + 1],
                    scalar2=0.0, op0=AL.add, op1=AL.max)

                # L2: 4 single psums; E relu+bias into pair tiles
                E0t = enc_p.tile([128, 2 * TB], BF16, tag=f"E0_{t % 3}")
                E1t = enc_p.tile([128, 2 * TB], BF16, tag=f"E1_{t % 3}")
                Epr = [E0t, E1t]
                for jj in range(4):
                    ep = phe.tile([128, TB], F32, tag="he")
                    for kk in range(2):
                        g = 2 * jj + kk
                        nc.tensor.matmul(
                            ep, CB[:, W2_C + 128 * g:W2_C + 128 * (g + 1)],
                            h1t[g], start=(kk == 0), stop=(kk == 1))
                    dst = Epr[jj // 2][:, TB * (jj % 2):TB * (jj % 2 + 1)]
                    bias = CF[:, B2SB_C + jj:B2SB_C + jj + 1]
                    if jj < 2:
                        nc.scalar.activation(out=dst, in_=ep, func=AF.Relu,
                                             bias=bias, scale=1.0)
                    else:
                        nc.vector.tensor_scalar(
                            out=dst, in0=ep, scalar1=bias, scalar2=0.0,
                            op0=AL.add, op1=AL.max)

                # scores: P = E*sr (Pool), 4 matmuls into quad S4[32k:]
                srs = sr[:, :]
                srb = bass.AP(tensor=srs.tensor, offset=srs.offset,
                              ap=[srs.ap[0], [0, 2], srs.ap[1]])
                P_pr = []
                for e in range(2):
                    pjp = work_p.tile([128, 2 * TB], BF16, tag=f"P{e}")
                    nc.gpsimd.tensor_mul(
                        pjp.rearrange("p (k n) -> p k n", k=2),
                        Epr[e].rearrange("p (k n) -> p k n", k=2), srb)
                    P_pr.append(pjp)
                kk = t % 2
                if kk == 0:
                    S2n = pq.tile([64, TB], F32, tag="q", name="S2")
                    _st['S2'] = S2n
                S2 = _st['S2']
                for jj in range(4):
                    nc.tensor.matmul(
                        S2[32 * kk:32 * kk + 32],
                        CB[:, SCORE_C + 32 * jj:SCORE_C + 32 * (jj + 1)],
                        P_pr[jj // 2][:, TB * (jj % 2):TB * (jj % 2 + 1)],
                        start=(jj == 0), stop=(jj == 3))
                return Epr

            def pair_tail(q, k2, Eprs):
                # C for the two tiles of this pair into C2[64kk:]
                C2 = pcv.tile([128, TB], F32, tag="cv")
                wt2 = _st['wt2']
                for kk in range(2):
                    k = kk
                    Epr = Eprs[kk]
                    P2_pr = []
                    for e in range(2):
                        p2 = work_p.tile([128, 2 * TB], BF16, tag=f"P2{e}")
                        for h in range(2):
                            jj = 2 * e + h
                            wrp = pw.tile([128, TB], F32, tag="w")
                            c0 = REPW4_C + 128 * (4 * k + jj)
                            nc.tensor.matmul(
                                wrp, CB[32 * k:32 * k + 32, c0:c0 + 128],
                                wt2[32 * k:32 * k + 32, :],
                                start=True, stop=True)
                            nc.vector.tensor_mul(
                                p2[:, TB * h:TB * (h + 1)],
                                Epr[e][:, TB * h:TB * (h + 1)], wrp)
                        P2_pr.append(p2)
                    for jj in range(4):
                        nc.tensor.matmul(
                            C2[64 * kk:64 * kk + 64],
                            CB[:, REPC_C + 64 * jj:REPC_C + 64 * (jj + 1)],
                            P2_pr[jj // 2][:, TB * (jj % 2):TB * (jj % 2 + 1)],
                            start=(jj == 0), stop=(jj == 3))

                # var (natural) + Newton rsqrt, batched over the pair
                sqb = work_p.tile([128, TB], BF16, tag="sqb")
                nc.scalar.activation(out=sqb, in_=C2, func=AF.Square)
                rn = work_p.tile([128, TB], BF16, tag="rn")
                nc.scalar.activation(out=rn, in_=C2, func=AF.Relu)
                vn = pcv.tile([128, 16], F32, tag="cv")
                for kk in range(2):
                    for s in range(NSUB):
                        nc.tensor.matmul(
                            vn[:, 8 * kk + 2 * s:8 * kk + 2 * s + 2],
                            sqb[64 * kk:64 * kk + 48, 128 * s:128 * (s + 1)],
                            CB[64 * kk:64 * kk + 48, SQONES_C:SQONES_C + 2],
                            start=True, stop=True)
                ve = work_p.tile([128, 16], F32, tag="ve")
                nc.vector.tensor_scalar(out=ve, in0=vn, scalar1=EPS,
                                        scalar2=None, op0=AL.add)
                veu = ve.bitcast(U32)
                y0b = work_p.tile([128, 16], U32, tag="y0b")
                nc.vector.tensor_scalar(out=y0b, in0=veu, scalar1=1,
                                        scalar2=None,
                                        op0=AL.logical_shift_right)
                mgs = CF[:, MAGIC_C:MAGIC_C + 1].bitcast(U32)
                mgb = bass.AP(tensor=mgs.tensor, offset=mgs.offset,
                              ap=[mgs.ap[0], [0, 16]])
                nc.gpsimd.tensor_sub(out=y0b, in0=mgb, in1=y0b)
                y0 = y0b.bitcast(F32)
                tt = work_p.tile([128, 16], F32, tag="tt")
                nc.gpsimd.tensor_tensor(out=tt, in0=ve, in1=y0, op=AL.mult)
                nc.gpsimd.tensor_tensor(out=tt, in0=tt, in1=y0, op=AL.mult)
                nc.vector.tensor_scalar(out=tt, in0=tt, scalar1=-0.5,
                                        scalar2=1.5, op0=AL.mult, op1=AL.add)
                rstdn = work_p.tile([128, 16], BF16, tag="rstdn")
                nc.gpsimd.tensor_tensor(out=rstdn, in0=y0, in1=tt,
                                        op=AL.mult)

                rT = pcv.tile([2, 2 * TB], BF16, tag="cv")
                for kk in range(2):
                    for s in range(NSUB):
                        nc.tensor.transpose(
                            rT[:, 512 * kk + 128 * s:512 * kk + 128 * (s + 1)],
                            rstdn[:, 8 * kk + 2 * s:8 * kk + 2 * s + 2], eyeb)
                rstdt = work_p.tile([3, 2 * TB], BF16, tag="rstdt")
                nc.gpsimd.memset(rstdt, 1.0)
                nc.vector.tensor_copy(out=rstdt[0:2, :], in_=rT)
                rrep = pcv.tile([128, TB], F32, tag="cv")
                for kk in range(2):
                    nc.tensor.matmul(
                        rrep[64 * kk:64 * kk + 64],
                        CB[0:3, REP3_C:REP3_C + 64],
                        rstdt[:, 512 * kk:512 * (kk + 1)],
                        start=True, stop=True)
                msb = work_p.tile([128, TB], BF16, tag="msb")
                nc.vector.tensor_tensor(out=msb, in0=rn, in1=rrep,
                                        op=AL.mult)
                return msb

            def pair_mlp(pp_, msb):
                h1f2 = pq.tile([64, TB], F32, tag="q")
                for k in range(2):
                    nc.tensor.matmul(
                        h1f2[32 * k:32 * k + 32],
                        CB[64 * k:64 * k + 48,
                           M1_C + 32 * k:M1_C + 32 * k + 32],
                        msb[64 * k:64 * k + 48, :],
                        start=True, stop=True)
                hh1 = work_p.tile([64, TB], BF16, tag="hh1")
                if prelu == 'dve':
                    lb1 = work_p.tile([64, TB], F32, tag="lb1")
                    nc.vector.tensor_scalar(
                        out=lb1, in0=h1f2, scalar1=CF[0:64, B1T_C:B1T_C + 1],
                        scalar2=None, op0=AL.add)
                    lt1 = work_p.tile([64, TB], BF16, tag="lt1")
                    nc.vector.tensor_scalar(out=lt1, in0=lb1, scalar1=0.01,
                                            scalar2=None, op0=AL.mult)
                    nc.vector.tensor_tensor(out=hh1, in0=lt1, in1=lb1,
                                            op=AL.max)
                else:
                    nc.scalar.activation(out=hh1, in_=h1f2, func=prelu_fn,
                                         bias=CF[0:64, B1T_C:B1T_C + 1],
                                         scale=1.0, alpha=0.01)
                h2f2 = pw.tile([64, TB], F32, tag="w")
                for k in range(2):
                    nc.tensor.matmul(
                        h2f2[32 * k:32 * k + 32],
                        CB[32 * k:32 * k + 32,
                           M2_C + 32 * k:M2_C + 32 * k + 32],
                        hh1[32 * k:32 * k + 32, :], start=True, stop=True)
                hh2 = work_p.tile([64, TB], BF16, tag="hh2")
                if prelu == 'dve':
                    lb2 = work_p.tile([64, TB], F32, tag="lb2")
                    nc.vector.tensor_scalar(
                        out=lb2, in0=h2f2, scalar1=CF[0:64, B2T_C:B2T_C + 1],
                        scalar2=None, op0=AL.add)
                    lt2 = work_p.tile([64, TB], BF16, tag="lt2")
                    nc.vector.tensor_scalar(out=lt2, in0=lb2, scalar1=0.01,
                                            scalar2=None, op0=AL.mult)
                    nc.vector.tensor_tensor(out=hh2, in0=lt2, in1=lb2,
                                            op=AL.max)
                else:
                    nc.scalar.activation(out=hh2, in_=h2f2, func=prelu_fn,
                                         bias=CF[0:64, B2T_C:B2T_C + 1],
                                         scale=1.0, alpha=0.01)
                of2 = pw.tile([64, TB], F32, tag="w")
                for k in range(2):
                    nc.tensor.matmul(
                        of2[32 * k:32 * k + 32],
                        CB[32 * k:32 * k + 32,
                           M3_C + 32 * k:M3_C + 32 * k + 32],
                        hh2[32 * k:32 * k + 32, :], start=True, stop=True)
                osb = work_p.tile([64, TB], F32, tag="osb")
                nc.scalar.activation(out=osb, in_=of2, func=AF.Tanh,
                                     bias=CF[0:64, B3T_C:B3T_C + 1],
                                     scale=1.0)
                for k in range(2):
                    t = 2 * pp_ + k
                    nc.gpsimd.tensor_copy(
                        out=gs[0:1, 8:12], in_=osb[32 * k:32 * k + 1, 508:512])
                    nc.gpsimd.dma_start(out=out[:, TB * t:TB * (t + 1)],
                                        in_=osb[32 * k:32 * k + 2, :])

            prelu_fn = AF.Prelu if prelu == 'act' else AF.Lrelu
            assert nt % 2 == 0
            for pp_ in range(nt // 2):
                Eq = []
                for k in range(2):
                    Eq_ = stage_a(2 * pp_ + k)
                    Eq.append(Eq_)
                wt2 = work_p.tile([64, TB], BF16, tag="wt2")
                nc.scalar.activation(out=wt2, in_=_st['S2'], func=AF.Exp,
                                     scale=0.25)
                _st['wt2'] = wt2
                msb_ = pair_tail(pp_, 0, Eq)
                pair_mlp(pp_, msb_)

    if split_waits:
        _split_multi_waits(nc)
    return nc


def kernel(**inputs):
    inputs = {k: np.asarray(v, np.float32) for k, v in inputs.items()}
    cb, cf = _pack_consts(inputs)

    if 'nc' not in _BASS_CACHE:
        _BASS_CACHE['nc'] = _build_bass()
    nc = _BASS_CACHE['nc']

    s = np.ascontiguousarray(inputs['s_input'])
    in_maps = []
    for i in range(N_CORES):
        xt = np.ones((128, BC), np.float32)
        xt[0:127] = s[i * BC:(i + 1) * BC].T
        in_maps.append({
            "s_in": xt.astype(ml_dtypes.bfloat16),
            "cb": cb,
            "cf": cf,
        })
    _BASS_CACHE['in_maps'] = in_maps
    res = run_bass_kernel_spmd(nc, in_maps, core_ids=list(range(N_CORES)))
    outs = []
    for i in range(N_CORES):
        o = np.asarray(res.results[i]["out"])           # [2, BC]
        outs.append(np.ascontiguousarray(o.T))
    return np.concatenate(outs, axis=0)
